# revision 5
# baseline (speedup 1.0000x reference)
"""Decoder-only attention block (QKV proj + MHA + out proj) on 8 TRN2 cores.

Sharding: core c -> (batch b = c//4, head-group g = c%4). Tensor-parallel over
heads (4 of 16 heads per core), data-parallel over batch (2). Each core
computes a partial c_proj over its 512 input features; host reduces the 4
partials per batch and adds biases.

Self-contained: hardcodes B=2, S=2048, D=2048, H=16.
"""

import os

import numpy as np

import concourse.bass as bass
import concourse.bacc as bacc
import concourse.tile as tile
from concourse import mybir
import concourse.bass_utils as bass_utils
from concourse.bass_interp import get_hw_module

B, S, D = 2, 2048, 2048
H, DH = 16, 128
N_CORES = 8
HL = H // 4            # 4 heads per core
FL = HL * DH           # 512 local features per core
KT = D // 128          # 16 contraction tiles
TT = S // 128          # 16 token tiles
QB = S // 512          # 4 token blocks
SCALE = 1.0 / float(np.sqrt(DH))

F16 = mybir.dt.float16
F32 = mybir.dt.float32

# Stash of the last BassKernelResults (for the local test harness only).
LAST_RESULTS = None
_PROG_CACHE = {}


def _build_program():

    from concourse.masks import make_identity

    nc = bacc.Bacc("TRN2", target_bir_lowering=False, debug=False,
                   num_devices=N_CORES)

    xt_d = nc.dram_tensor("xt", [D, S], F16, kind="ExternalInput")
    wqk_d = nc.dram_tensor("wqk", [D, 2 * FL], F16, kind="ExternalInput")
    wv_d = nc.dram_tensor("wv", [D, FL], F16, kind="ExternalInput")
    wp_d = nc.dram_tensor("wp", [FL, D], F16, kind="ExternalInput")
    bqk_d = nc.dram_tensor("bqk", [128, 8], F32, kind="ExternalInput")
    kb_d = nc.dram_tensor("kb", [128, KT], F32, kind="ExternalInput")
    out_d = nc.dram_tensor("out", [S, D], F32, kind="ExternalOutput")

    xt_ap, wqk_ap, wv_ap, wp_ap = xt_d.ap(), wqk_d.ap(), wv_d.ap(), wp_d.ap()
    bqk_ap, kb_ap, out_ap = bqk_d.ap(), kb_d.ap(), out_d.ap()

    with tile.TileContext(nc) as tc, tc.tile_pool(name="pers", bufs=1) as pers:
        # ---- persistent tiles (live across phases) ----
        qt = [pers.tile([128, S], F16, tag=f"qt{h}", name=f"qt{h}") for h in range(HL)]
        ktt = [pers.tile([128, S], F16, tag=f"kt{h}", name=f"ktt{h}") for h in range(HL)]
        ot = [pers.tile([128, S], F16, tag=f"ot{h}", name=f"ot{h}") for h in range(HL)]
        vaug = [[pers.tile([128, DH + 1], F16, tag=f"v{t}_{h}", name=f"v{t}_{h}")
                 for h in range(HL)] for t in range(TT)]
        wp_sb = [pers.tile([128, D], F16, tag=f"wp{h}", name=f"wp{h}") for h in range(HL)]
        bqk_sb = pers.tile([128, 8], F32, tag="bqk", name="bqk_sb")
        kb_sb = pers.tile([128, KT], F32, tag="kb", name="kb_sb")
        ident = pers.tile([128, 128], F16, tag="ident", name="ident")

        nc.sync.dma_start(bqk_sb[:], bqk_ap[:])
        nc.sync.dma_start(kb_sb[:], kb_ap[:])
        for h in range(HL):
            nc.sync.dma_start(wp_sb[h][:], wp_ap[h * 128:(h + 1) * 128, :])
        make_identity(nc, ident[:])
        for t in range(TT):
            for h in range(HL):
                nc.vector.memset(vaug[t][h][:, DH:DH + 1], 1.0)

        # ---- phase 1: QKV projection ----
        with (
            tc.tile_pool(name="p1in", bufs=1) as p1in,
            tc.tile_pool(name="p1ps", bufs=1, space="PSUM") as p1ps,
        ):
            xt_sb, wqk_sb, wv_sb = [], [], []
            for kt in range(KT):
                w = p1in.tile([128, 2 * FL], F16, tag=f"wqk{kt}", name=f"wqk{kt}")
                nc.sync.dma_start(w[:], wqk_ap[kt * 128:(kt + 1) * 128, :])
                wqk_sb.append(w)
                x = p1in.tile([128, S], F16, tag=f"xt{kt}", name=f"xt{kt}")
                nc.sync.dma_start(x[:], xt_ap[kt * 128:(kt + 1) * 128, :])
                xt_sb.append(x)
                v = p1in.tile([128, FL], F16, tag=f"wv{kt}", name=f"wv{kt}")
                nc.sync.dma_start(v[:], wv_ap[kt * 128:(kt + 1) * 128, :])
                wv_sb.append(v)

            # Q^T and K^T: [feat, tok], weight chunks stationary
            for f in range(8):
                ps = [None] * QB
                for kt in range(KT):
                    for tb in range(QB):
                        if kt == 0:
                            ps[tb] = p1ps.tile([128, 512], F32, tag=f"psqk{tb}", bufs=1, name=f"psqk{tb}")
                        nc.tensor.matmul(
                            ps[tb][:],
                            wqk_sb[kt][:, f * 128:(f + 1) * 128],
                            xt_sb[kt][:, tb * 512:(tb + 1) * 512],
                            start=(kt == 0), stop=(kt == KT - 1),
                            skip_group_check=True,
                        )
                dest = qt[f] if f < HL else ktt[f - HL]
                for tb in range(QB):
                    nc.scalar.add(dest[:, tb * 512:(tb + 1) * 512], ps[tb][:],
                                  bqk_sb[:, f:f + 1])

            # V: [tok, feat], x^T chunks stationary
            for t in range(TT):
                psv = p1ps.tile([128, FL], F32, tag="psv", bufs=2, name="psv")
                for kt in range(KT):
                    nc.tensor.matmul(
                        psv[:],
                        xt_sb[kt][:, t * 128:(t + 1) * 128],
                        wv_sb[kt][:],
                        start=(kt == 0), stop=(kt == KT - 1),
                    )
                for h in range(HL):
                    nc.vector.tensor_copy(vaug[t][h][:, 0:DH],
                                          psv[:, h * 128:(h + 1) * 128])

        # ---- phase 2: attention per (token-block, head) ----
        with (
            tc.tile_pool(name="p2", bufs=1) as p2,
            tc.tile_pool(name="p2ps", bufs=1, space="PSUM") as p2ps,
        ):
            for qb in range(QB):
                for h in range(HL):
                    # scores^T [ktok, qtok] -> exp -> E^T (f16)
                    e_tiles = []
                    for kt in range(KT):
                        pss = p2ps.tile([128, 512], F32, tag="pss", bufs=2, name="pss")
                        nc.tensor.matmul(
                            pss[:],
                            ktt[h][:, kt * 128:(kt + 1) * 128],
                            qt[h][:, qb * 512:(qb + 1) * 512],
                            start=True, stop=True,
                        )
                        e = p2.tile([128, 512], F16, tag=f"e{kt}", bufs=2, name=f"e{kt}")
                        nc.scalar.activation(
                            e[:], pss[:], mybir.ActivationFunctionType.Exp,
                            bias=kb_sb[:, kt:kt + 1], scale=SCALE,
                        )
                        e_tiles.append(e)
                    # O = A_unnorm @ [V | 1]; col DH is the softmax denom
                    for j in range(4):
                        pso = p2ps.tile([128, DH + 1], F32, tag="pso", bufs=2, name="pso")
                        for kt in range(KT):
                            nc.tensor.matmul(
                                pso[:],
                                e_tiles[kt][:, j * 128:(j + 1) * 128],
                                vaug[kt][h][:],
                                start=(kt == 0), stop=(kt == KT - 1),
                            )
                        rec = p2.tile([128, 1], F32, tag="rec", bufs=4, name="rec")
                        nc.vector.reciprocal(rec[:], pso[:, DH:DH + 1])
                        osb = p2.tile([128, 128], F16, tag="osb", bufs=4, name="osb")
                        nc.vector.tensor_scalar_mul(osb[:], pso[:, 0:DH], rec[:])
                        pst = p2ps.tile([128, 128], F16, tag="pst", bufs=2, name="pst")
                        nc.tensor.transpose(pst[:], osb[:], ident[:])
                        tok = qb * 512 + j * 128
                        nc.vector.tensor_copy(ot[h][:, tok:tok + 128], pst[:])

        # ---- phase 3: partial c_proj [tok, dout] ----
        with (
            tc.tile_pool(name="p3", bufs=1) as p3,
            tc.tile_pool(name="p3ps", bufs=1, space="PSUM") as p3ps,
        ):
            for t in range(TT):
                pps = [p3ps.tile([128, 512], F32, tag=f"psp{nb}", bufs=2, name=f"psp{nb}")
                       for nb in range(4)]
                for h in range(HL):
                    for nb in range(4):
                        nc.tensor.matmul(
                            pps[nb][:],
                            ot[h][:, t * 128:(t + 1) * 128],
                            wp_sb[h][:, nb * 512:(nb + 1) * 512],
                            start=(h == 0), stop=(h == HL - 1),
                            skip_group_check=True,
                        )
                for nb in range(4):
                    st = p3.tile([128, 512], F32, tag="stage", bufs=4, name="stage")
                    nc.scalar.copy(st[:], pps[nb][:])
                    nc.sync.dma_start(
                        out_ap[t * 128:(t + 1) * 128,
                               nb * 512:(nb + 1) * 512], st[:])

    nc.compile()
    nc.m = get_hw_module(nc.m)
    return nc


def kernel(hidden_states, attention_mask, w_attn, b_attn, w_proj, b_proj):
    global LAST_RESULTS
    hidden_states = np.asarray(hidden_states, dtype=np.float32)
    attention_mask = np.asarray(attention_mask, dtype=np.float32)
    w_attn = np.asarray(w_attn, dtype=np.float32)
    b_attn = np.asarray(b_attn, dtype=np.float32)
    w_proj = np.asarray(w_proj, dtype=np.float32)
    b_proj = np.asarray(b_proj, dtype=np.float32)

    if "prog" not in _PROG_CACHE:
        _PROG_CACHE["prog"] = _build_program()
    nc = _PROG_CACHE["prog"]

    in_maps = []
    for c in range(N_CORES):
        b, g = divmod(c, 4)
        xt = np.ascontiguousarray(hidden_states[b].T).astype(np.float16)
        wq = w_attn[:, g * FL:(g + 1) * FL]
        wk = w_attn[:, D + g * FL:D + (g + 1) * FL]
        wv = w_attn[:, 2 * D + g * FL:2 * D + (g + 1) * FL]
        wqk = np.concatenate([wq, wk], axis=1).astype(np.float16)
        wp = np.ascontiguousarray(w_proj[g * FL:(g + 1) * FL, :]).astype(
            np.float16)
        bq = b_attn[g * FL:(g + 1) * FL]
        bk = b_attn[D + g * FL:D + (g + 1) * FL]
        bqk = np.ascontiguousarray(
            np.concatenate([bq, bk]).reshape(8, 128).T).astype(np.float32)
        kb = np.ascontiguousarray(
            ((1.0 - attention_mask[b]) * -10000.0).reshape(KT, 128).T
        ).astype(np.float32)
        in_maps.append({
            "xt": xt,
            "wqk": wqk,
            "wv": np.ascontiguousarray(wv).astype(np.float16),
            "wp": wp,
            "bqk": bqk,
            "kb": kb,
        })

    if not os.environ.get("KERNEL_ALLOW_TRACE"):
        os.environ["BASS_NEVER_TRACE"] = "1"
    res = bass_utils.run_bass_kernel_spmd(nc, in_maps, list(range(N_CORES)))
    LAST_RESULTS = res

    # host reduce: sum the 4 head-group partials per batch, add biases.
    # V-bias contribution: rows of A sum to 1, so each core's O gains b_v
    # per row; through c_proj that's a constant row b_v @ w_proj_slice.
    out = np.zeros((B, S, D), dtype=np.float32)
    for c in range(N_CORES):
        b, g = divmod(c, 4)
        out[b] += res.results[c]["out"]
    bias_row = b_proj.astype(np.float64).copy()
    for g in range(4):
        bv = b_attn[2 * D + g * FL:2 * D + (g + 1) * FL].astype(np.float64)
        bias_row += bv @ w_proj[g * FL:(g + 1) * FL, :].astype(np.float64)
    out += bias_row.astype(np.float32)[None, None, :]
    return out
